# revision 14
# baseline (speedup 1.0000x reference)
"""Bidirectional leaky-ESN (B=8,T=2048,D=64,H=1024,O=16) on 8 TRN2 NeuronCores.

Strategy
--------
The recurrence  h_t = 0.1 h_{t-1} + 0.9 tanh(u_proj_t + h_{t-1} W^T)  is a
contraction (leak 0.9, spectral radius 0.9; measured decay ~0.56/step), so
time can be chunked with a short washout: each of the 2 directions x 8
batches is split into C=64 chunks of L=32 steps; every chunk is run
independently from state 0 starting WASH=12 steps early.  Initial-condition
error decays below the bf16 compute floor (~2e-4 vs ~3.4e-3 measured in
simulation against an fp64 oracle).

This turns 2*2048 serial steps into L+WASH=44 steps over 1024 parallel
sequences.  Sharding: cores 0-3 forward direction (batches 2k,2k+1),
cores 4-7 backward - 128 sequences per core = full PE partition width,
single w_out section per core.

Formulation: with s := h/0.9 (folds the 0.9 into W' = 0.9 W and
w_out'' = 0.9 w_out), the state kept on-chip is the pre-activation
pre_k (PSUM, fp32) and z_k = tanh(pre_k) (SBUF, bf16):

    pre_{k+1} = (u_proj_{k+1} - 0.1 u_proj_k) + 0.1 pre_k + W' z_k

The u_proj difference is folded host-side into the streamed input v; the
0.1 pre_k term re-enters PSUM through an identity matmul of d_k = 0.1*pre_k
(one VectorE tensor_scalar per tile, PSUM->SBUF bf16).  Every non-matmul
instruction therefore has PE as its only semaphore dependency (the
TT/STT ISA structs only fit one sync wait - this layout is wait-minimal).

Layout is transposed (H on partitions: 8 tiles [128,128], sequences on the
free dim).  Per step: 8 identity-matmuls + 64 W'-stationary matmuls + 8
u-injection matmuls (K=65, w_in|w_bias augmented) accumulate PSUM; 8 ACT
tanh -> z_store, 8 DVE scales -> d.  All z are stored (44 slots); a batched
readout phase computes q'_k = w_out''^T z_k, and the host runs the exact
geometric scan  r_k = 0.1 r_{k-1} + q'_k  (= w_out^T h_k), reassembling
fwd+bwd+bias into [B,T,O].
"""

import numpy as np
import ml_dtypes

bf16 = ml_dtypes.bfloat16

B, T, D, H, O = 8, 2048, 64, 1024, 16
A = 0.9           # leaky rate
C = 64            # chunks per (batch, direction)
L = T // C        # 32 steps of real output per chunk
WASH = 12         # washout steps
STEPS = L + WASH
NCORES = 8
NI = H // 128     # 8 partition tiles of H
KAUG = D + 1      # 65: input dim + bias indicator row

_cached = {}


def _build_program():
    import concourse.bacc as bacc
    import concourse.mybir as mybir
    from concourse.tile import TileContext

    dt = mybir.dt
    nc = bacc.Bacc(trn_type="TRN2", target_bir_lowering=False, debug=False)

    wT_d = nc.dram_tensor("wT", [H, H], dt.bfloat16, kind="ExternalInput").ap()
    winT_d = nc.dram_tensor("winT", [KAUG, H], dt.bfloat16, kind="ExternalInput").ap()
    woutT_d = nc.dram_tensor("woutT", [128, NI * O], dt.bfloat16, kind="ExternalInput").ap()
    ident_d = nc.dram_tensor("ident", [128, 128], dt.bfloat16, kind="ExternalInput").ap()
    vbuf_d = nc.dram_tensor("vbuf", [KAUG, STEPS * 128], dt.bfloat16, kind="ExternalInput").ap()
    qout_d = nc.dram_tensor("qout", [O, STEPS * 128], dt.float32, kind="ExternalOutput").ap()

    with TileContext(nc) as tc:
        _body(tc, mybir, wT_d, winT_d, woutT_d, ident_d, vbuf_d, qout_d)
    nc.compile()
    return nc


def _body(tc, mybir, wT_d, winT_d, woutT_d, ident_d, vbuf_d, qout_d):
    dt = mybir.dt
    nc = tc.nc
    Tanh = mybir.ActivationFunctionType.Tanh

    with (
        tc.tile_pool(name="const", bufs=1) as constp,
        tc.tile_pool(name="dp", bufs=4) as dp,
        tc.tile_pool(name="zstore", bufs=1) as zstorep,
        tc.tile_pool(name="stage", bufs=1) as stagep,
        tc.tile_pool(name="pre", bufs=1, space="PSUM") as prep,
    ):
        # ---- prologue: load weights + all per-step inputs ----
        wT_sb = []
        for j in range(NI):
            t = constp.tile([128, H], dt.bfloat16, tag=f"wT{j}", name=f"wT{j}")
            nc.sync.dma_start(t[:], wT_d[j * 128:(j + 1) * 128, :])
            wT_sb.append(t)
        winT_sb = constp.tile([KAUG, H], dt.bfloat16, tag="winT", name="winT")
        nc.sync.dma_start(winT_sb[:], winT_d[:])
        woutT_sb = constp.tile([128, NI * O], dt.bfloat16, tag="woutT", name="woutT")
        nc.sync.dma_start(woutT_sb[:], woutT_d[:])
        ident_sb = constp.tile([128, 128], dt.bfloat16, tag="ident", name="ident")
        nc.sync.dma_start(ident_sb[:], ident_d[:])
        vbuf_sb = constp.tile([KAUG, STEPS * 128], dt.bfloat16, tag="vbuf", name="vbuf")
        nc.sync.dma_start(vbuf_sb[:], vbuf_d[:])

        z_store = [zstorep.tile([128, STEPS * 128], dt.bfloat16, tag=f"zs{i}", name=f"zs{i}")
                   for i in range(NI)]
        stage_sb = stagep.tile([O, STEPS * 128], dt.float32, tag="stage", name="stage")

        # ---- serial recurrence, all 128 sequences in lockstep ----
        d_prev = None
        z_prev = None
        for k in range(STEPS):
            vk = vbuf_sb[:, k * 128:(k + 1) * 128]
            z_cur = [z_store[i][:, k * 128:(k + 1) * 128] for i in range(NI)]
            d_cur = [dp.tile([128, 128], dt.bfloat16, tag=f"d{i}", name=f"d{i}_{k}")
                     for i in range(NI)]
            for i in range(NI):
                pre = prep.tile([128, 128], dt.float32, tag=f"pre{i}", name=f"pre{i}_{k}")
                if k == 0:
                    nc.tensor.matmul(pre, winT_sb[:, i * 128:(i + 1) * 128], vk,
                                     start=True, stop=True)
                else:
                    # pre = 0.1*pre_prev (via identity) + sum_j W'T_j z_prev_j + v_k
                    nc.tensor.matmul(pre, ident_sb[:], d_prev[i], start=True, stop=False)
                    for j in range(NI):
                        nc.tensor.matmul(pre, wT_sb[j][:, i * 128:(i + 1) * 128],
                                         z_prev[j], start=False, stop=False)
                    nc.tensor.matmul(pre, winT_sb[:, i * 128:(i + 1) * 128], vk,
                                     start=False, stop=True)
                nc.scalar.activation(z_cur[i], pre, Tanh)
                if k + 1 < STEPS:
                    # ScalarE too (not DVE): a second reader engine on the same
                    # PSUM bank would be serialized against the tanh anyway
                    nc.scalar.mul(d_cur[i], pre, 0.1)
            d_prev = d_cur
            z_prev = z_cur

        # ---- batched readout: q'[:, k*128+s] = w_out''^T z_k ----
        for g in range((STEPS + 3) // 4):
            nslot = min(4, STEPS - g * 4)
            pr = prep.tile([O, 512], dt.float32, tag=f"pre{g % NI}", name=f"pr_{g}")
            for mm in range(nslot):
                k = g * 4 + mm
                for i in range(NI):
                    nc.tensor.matmul(pr[:, mm * 128:(mm + 1) * 128],
                                     woutT_sb[:, i * O:(i + 1) * O],
                                     z_store[i][:, k * 128:(k + 1) * 128],
                                     start=(i == 0), stop=(i == NI - 1))
            # ScalarE (not DVE): keeps every cross-engine dep on one semaphore
            nc.scalar.copy(stage_sb[:, g * 512:g * 512 + nslot * 128], pr[:, :nslot * 128])
        nc.sync.dma_start(qout_d[:], stage_sb[:])


def _prep_inputs(u, w, w_in, w_bias, w_out):
    """Host-side prep: per-core input maps (bf16 except the f32 output)."""
    wT = np.ascontiguousarray((A * w).T).astype(bf16)                     # [j, i]
    winT = np.ascontiguousarray(
        np.concatenate([w_in, w_bias[:, None]], axis=1).T).astype(bf16)   # [65, H]
    ident = np.eye(128, dtype=np.float32).astype(bf16)
    in_maps = []
    for core in range(NCORES):
        d = core // 4                       # 0 fwd, 1 bwd
        w2 = (A * w_out[1 + d * H:1 + (d + 1) * H, :]).astype(np.float32)  # [H, O]
        woutT = np.ascontiguousarray(
            w2.reshape(NI, 128, O).transpose(1, 0, 2).reshape(128, NI * O)).astype(bf16)
        v = np.zeros((STEPS, KAUG, 128), np.float32)
        ks = np.arange(STEPS)
        for b_loc in range(2):
            b = 2 * (core % 4) + b_loc
            ud = u[b] if d == 0 else u[b, ::-1]
            for c in range(C):
                ts = c * L - WASH + ks
                valid = ts >= 0
                s_idx = b_loc * C + c
                v[valid, :D, s_idx] = ud[ts[valid]]
                v[valid, D, s_idx] = 1.0
        v[1:] -= 0.1 * v[:-1]                # fold the leak into the input stream
        vbuf = np.ascontiguousarray(
            v.transpose(1, 0, 2).reshape(KAUG, STEPS * 128)).astype(bf16)
        in_maps.append({"wT": wT, "winT": winT, "woutT": woutT,
                        "ident": ident, "vbuf": vbuf})
    return in_maps


def _assemble(results, w_out):
    y = np.zeros((B, T, O), np.float32)
    dec = 0.1 ** np.arange(STEPS, dtype=np.float64)
    for core in range(NCORES):
        q = np.asarray(results[core]["qout"], np.float32).reshape(O, STEPS, 128)
        d = core // 4
        # exact geometric scan over steps: r_k = sum_j 0.1^j q'_{k-j}
        r = np.zeros((O, STEPS, 128), np.float32)
        acc = np.zeros((O, 128), np.float32)
        for k in range(STEPS):
            acc = 0.1 * acc + q[:, k]
            r[:, k] = acc
        r = r[:, WASH:]                                   # [O, L, 128]
        for b_loc in range(2):
            b = 2 * (core % 4) + b_loc
            qq = r[:, :, b_loc * C:(b_loc + 1) * C]       # [O, L(m), C(c)]
            tmp = qq.transpose(2, 1, 0).reshape(T, O)     # t = c*L + m
            if d == 0:
                y[b] += tmp
            else:
                y[b, ::-1] += tmp
    y += w_out[0][None, None, :].astype(np.float32)
    return y


def kernel(u, w, w_in, w_bias, w_out):
    from concourse.bass_utils import run_bass_kernel_spmd

    u = np.asarray(u, np.float32)
    w = np.asarray(w, np.float32)
    w_in = np.asarray(w_in, np.float32)
    w_bias = np.asarray(w_bias, np.float32)
    w_out = np.asarray(w_out, np.float32)

    if "nc" not in _cached:
        _cached["nc"] = _build_program()
    nc = _cached["nc"]
    in_maps = _prep_inputs(u, w, w_in, w_bias, w_out)
    res = run_bass_kernel_spmd(nc, in_maps, list(range(NCORES)))
    return _assemble(res.results, w_out)
